# revision 15
# baseline (speedup 1.0000x reference)
"""Trainium2 Bass kernel for SAGAN-style spatial self-attention (B=4, C=256,
H=W=64): y = gamma * attention(x) + x with 1x1-conv q/k/v projections.

Sharding: 8 cores = 4 batch elements x 2 query-row halves; each core computes
its [2048, 256] slice of (gamma*attn)^T independently; the host adds x.

Division of labor: the tiny q/k projections (1.4% of FLOPs) run on host in
f32 and are shipped pre-striped for the PE row-tiling (q4 replicated into the
four 32-partition groups, k4 in row-tile chunk layout), so the
energy->exp->out pipeline starts as soon as ~1MB of q/k lands (~9us) instead
of waiting for the full 2MB x + projection chain (~27us). The v projection
(heavy input side) and all attention FLOPs (99%) stay on device.

Per-core algorithm (out = lhsT.T @ rhs on TensorE):
  - energy transposed, e^T[j, i] = k_chunk.T @ q, K=32 contraction row-tiled
    4x via tile_position so four j-chunks run concurrently; exp (ScalarE,
    [128, 4, 512] per group, the ~2us/group softmax floor) evacuates PSUM.
  - v^T = xf.T @ (gamma*Wv^T) with two ones-columns; the out accumulation
    then also produces the softmax denominator s_i for free.
  - out^T[i, c] accumulates over all 32 j-chunks in PSUM; epilogue multiplies
    by 1/s_i, DMAs bf16; host adds x + gamma*bv.
Schedule: the energy+exp stream for all 32 groups leads the out-matmul stream
by LEAD groups; v-projections are emitted under the exp shadow. The
v-projection PSUM pool closes before the out-accumulator pool opens so PSUM
stays within 8 banks (psE 4 + psA <= 4, then psE 4 + psO 4 = 8).
All matmuls bf16; PSUM/softmax/epilogue fp32.
"""

import sys

import numpy as np

for _p in ("/opt/trn_rl_repo", "/root/.axon_site", "/root/.axon_site/_ro/pypackages"):
    if _p not in sys.path:
        sys.path.insert(0, _p)

B, C, HW, N, D = 4, 256, 64, 4096, 32
NQ = N // 2
IBLK = 512
NIB = NQ // IBLK     # 4 i-blocks per core
NJ = N // 128        # 32 j-chunks
JG = 4               # j-chunks per group == row-tiling factor
NIC = IBLK // 128    # 4 i-chunks of 128 per i-block
NG = NIB * (NJ // JG)  # 32 groups
LEAD = 16            # energy/exp groups emitted ahead of out-matmuls

_NC_CACHE = {}


def _build_nc():
    import concourse.bass as bass
    import concourse.tile as tile
    from concourse import bacc, mybir

    f32 = mybir.dt.float32
    bf16 = mybir.dt.bfloat16
    Exp = mybir.ActivationFunctionType.Exp
    ts = bass.ts

    nc = bacc.Bacc(None, target_bir_lowering=False, debug=False)

    k4_d = nc.declare_dram_parameter("k4", [128, NJ // 4 * 128], bf16, isOutput=False)
    q4_d = nc.declare_dram_parameter("q4", [128, NQ], bf16, isOutput=False)
    xf_d = nc.declare_dram_parameter("xf", [C, N], bf16, isOutput=False)
    wvt_d = nc.declare_dram_parameter("wvt", [C, C], bf16, isOutput=False)
    out_d = nc.declare_dram_parameter("out", [NQ, C], bf16, isOutput=True)

    with tile.TileContext(nc) as tc:
        with (
            tc.tile_pool(name="const", bufs=1) as cpool,
            tc.tile_pool(name="sblk", bufs=LEAD + 2) as spool,
            tc.tile_pool(name="ytile", bufs=3) as ypool,
            tc.tile_pool(name="small", bufs=4) as rpool,
        ):
            prime_in = rpool.tile([1, 2], f32, tag="prime", name="prime_in")
            prime_out = rpool.tile([1, 2], f32, tag="prime", name="prime_out")
            nc.vector.memset(prime_in[:], 0.0)
            nc.scalar.activation(prime_out[:], prime_in[:], Exp)

            xf_sb = cpool.tile([128, 2, N], bf16)
            wv_sb = cpool.tile([128, 2, C], bf16)
            k4_sb = cpool.tile([128, NJ // 4, 128], bf16)
            q4_sb = cpool.tile([128, NQ], bf16)
            vt_sb = cpool.tile([128, NJ, C + 2], bf16)
            ones_sb = cpool.tile([128, NJ, 2], f32)

            # critical-path q/k striped tiles first, via scalar-engine HWDGE;
            # first pieces cover exactly the first four energy groups
            k4_r = k4_d[:].rearrange("p (s c) -> p s c", c=128)
            nc.scalar.dma_start(k4_sb[:, 0:4, :], k4_r[:, 0:4, :])
            nc.scalar.dma_start(q4_sb[:, 0:512], q4_d[:, 0:512])
            nc.scalar.dma_start(k4_sb[:, 4:8, :], k4_r[:, 4:8, :])
            nc.scalar.dma_start(q4_sb[:, 512:1024], q4_d[:, 512:1024])
            nc.scalar.dma_start(q4_sb[:, ts(1, 1024)], q4_d[:, ts(1, 1024)])
            for ec in range(2):
                nc.scalar.dma_start(wv_sb[:, ec, :], wvt_d[ts(ec, 128), :])
            # bulk x for the v-projection
            for p in range(N // 1024):
                for ec in range(2):
                    nc.gpsimd.dma_start(
                        xf_sb[:, ec, ts(p, 1024)], xf_d[ts(ec, 128), ts(p, 1024)]
                    )
            nc.vector.memset(ones_sb[:], 1.0)
            nc.vector.tensor_copy(vt_sb[:, :, C : C + 2], ones_sb[:])

            groups = [(ib, jg) for ib in range(NIB) for jg in range(NJ // JG)]
            opss = {}
            s_tiles = {}

            def emit_energy_exp(t):
                ib, jg = groups[t]
                eps = psE.tile([128, JG, IBLK], f32, tag="psE", name=f"eps{t}")
                for g in range(JG):
                    nc.tensor.matmul(
                        eps[:, g, :],
                        k4_sb[32 * g : 32 * (g + 1), jg, :],
                        q4_sb[32 * g : 32 * (g + 1), ts(ib, IBLK)],
                        start=True,
                        stop=True,
                        tile_position=(32 * g, 0),
                    )
                s_t = spool.tile([128, JG, IBLK], bf16, tag="sblk", name=f"s{t}")
                nc.scalar.activation(s_t[:], eps[:], Exp)
                s_tiles[t] = s_t

            def emit_out(t):
                ib, jg = groups[t]
                if jg == 0:
                    opss[ib] = [
                        psO.tile([128, C + 2], f32, tag="psO", name=f"ops{ib}_{i2}")
                        for i2 in range(NIC)
                    ]
                s_t = s_tiles.pop(t)
                for ic2 in range(NIC):
                    for g in range(JG):
                        j = jg * JG + g
                        nc.tensor.matmul(
                            opss[ib][ic2][:],
                            s_t[:, g, ts(ic2, 128)],
                            vt_sb[:, j, :],
                            start=(j == 0),
                            stop=(j == NJ - 1),
                        )
                if jg == NJ // JG - 1:
                    for ic2 in range(NIC):
                        ic = ib * NIC + ic2
                        ops = opss[ib][ic2]
                        r = rpool.tile([128, 1], f32, tag="small", name=f"r{ib}_{ic2}")
                        nc.vector.reciprocal(r[:], ops[:, C : C + 1])
                        y = ypool.tile([128, C], bf16, tag="ytile", name=f"y{ic}")
                        nc.vector.tensor_scalar_mul(y[:], ops[:, 0:C], r[:])
                        nc.sync.dma_start(out_d[ts(ic, 128), :], y[:])

            with tc.tile_pool(name="psE", bufs=1, space="PSUM") as psE:
                with tc.tile_pool(name="psA", bufs=3, space="PSUM") as psA:
                    # first energy+exp groups as soon as q4/k4 land
                    for t in range(4):
                        emit_energy_exp(t)
                    # v-projections under the exp shadow; later energies
                    # interleave as the energy PSUM buffer frees up
                    for j in range(NJ):
                        ps = psA.tile([128, C], f32, tag="psA", name=f"psv{j}")
                        for ec in range(2):
                            nc.tensor.matmul(
                                ps[:],
                                xf_sb[:, ec, ts(j, 128)],
                                wv_sb[:, ec, :],
                                start=(ec == 0),
                                stop=(ec == 1),
                            )
                        nc.vector.tensor_copy(vt_sb[:, j, 0:C], ps[:])
                        if j % 4 == 3 and 4 + j // 4 < LEAD:
                            emit_energy_exp(4 + j // 4)
                    for t in range(4 + NJ // 4, LEAD):
                        emit_energy_exp(t)
                # psA closed: out-accumulators may now use its banks
                with tc.tile_pool(name="psO", bufs=NIC, space="PSUM") as psO:
                    for t in range(NG):
                        if t + LEAD < NG:
                            emit_energy_exp(t + LEAD)
                        emit_out(t)

    nc.compile()
    return nc


def _get_nc():
    if "nc" not in _NC_CACHE:
        _NC_CACHE["nc"] = _build_nc()
    return _NC_CACHE["nc"]


def kernel(x, Wq, bq, Wk, bk, Wv, bv, gamma):
    import ml_dtypes
    from concourse.bass_utils import run_bass_kernel_spmd

    bf = ml_dtypes.bfloat16
    x = np.asarray(x, dtype=np.float32)
    gamma_v = float(np.asarray(gamma).reshape(-1)[0])
    xf = x.reshape(B, C, N)
    Wq32 = np.asarray(Wq, np.float32)
    Wk32 = np.asarray(Wk, np.float32)
    bq32 = np.asarray(bq, np.float32)
    bk32 = np.asarray(bk, np.float32)
    wvt = np.ascontiguousarray((gamma_v * np.asarray(Wv, np.float32).T).astype(bf))
    gbv = (gamma_v * np.asarray(bv, np.float32))[None, :]

    in_maps = []
    for core in range(8):
        b, h = divmod(core, 2)
        sl = slice(h * NQ, (h + 1) * NQ)
        xb = xf[b]
        xperm = np.ascontiguousarray(
            np.concatenate([xb[:, sl], xb[:, 0 : h * NQ], xb[:, (h + 1) * NQ :]], axis=1)
        )
        # host-side q/k projections in f32, pre-striped for PE row-tiling
        q = Wq32 @ xperm[:, 0:NQ] + bq32[:, None]   # [32, 2048]
        k = Wk32 @ xperm + bk32[:, None]            # [32, 4096]
        q4 = np.ascontiguousarray(np.tile(q, (4, 1)).astype(bf))  # [128, 2048]
        # k4[32g+d, jj*128+c] = k[d, 512jj+128g+c]
        k4 = np.ascontiguousarray(
            k.reshape(D, NJ // 4, 4, 128).transpose(2, 0, 1, 3).reshape(128, -1).astype(bf)
        )
        in_maps.append(
            {
                "xf": xperm.astype(bf),
                "q4": q4,
                "k4": k4,
                "wvt": wvt,
            }
        )

    nc = _get_nc()
    res = run_bass_kernel_spmd(nc, in_maps, core_ids=list(range(8)))
    y = np.empty((B, C, N), np.float32)
    for core in range(8):
        b, h = divmod(core, 2)
        sl = slice(h * NQ, (h + 1) * NQ)
        y[b][:, sl] = res.results[core]["out"].astype(np.float32).T + (xf[b][:, sl] + gbv.T)
    return y.reshape(B, C, HW, HW)
